# revision 14
# baseline (speedup 1.0000x reference)
"""AttentionUpscaling Trainium2 kernel.

Strategy (8 NeuronCores):
  - Pure data parallelism over batch (4) x query-half (2): each core owns one
    (batch, q-half) shard of the L x L attention matmul (the ~97 GFLOP that
    dominate this problem).
  - Host side (sharding prep): bilinear 2x upsample (exact jax semantics via a
    sparse banded matrix), unfold of the high-frequency residual, fp8e4m3
    quantization (attn pre-scaled by 2^17 into the fp8 sweet spot), and
    device-layout packing so every band is ONE contiguous-per-partition DMA.
  - Device side (SPMD bass/Tile program, same NEFF on all 8 cores):
    rec[q, d] = sum_m attnT[m, q] * hf[m, d]; contraction over m=4096 in 16
    DoubleRow chunks of 256 (fp8e4m3 at 2x PE rate), hf SBUF-resident (one
    3 MB DMA), attnT streamed in 256-column bands (one 1 MB DMA per band,
    double-buffered), PSUM accumulation, DVE copyback with fused fp32->bf16
    cast, HWDGE DMA out (bf16 halves the output traffic).
  - Host side (gather): overlap-add fold + overlap-count normalization (the
    2^17 attn pre-scale is folded into the normalization divide) + base image
    add, then stitch the two q-halves per batch.
"""

import os

import numpy as np

# ---------------------------------------------------------------- constants
B, C = 4, 3
HH = 512          # HR height/width
HL = 256          # LR height/width
K = 16            # HR patch size
S = 8             # HR stride
NH = (HH - K) // S + 1          # 63 patches per axis
L = NH * NH                     # 3969 patches
CKK = C * K * K                 # 768
NPH = 32                        # patch-rows per core (ph 0..31 / 31..62)
LQ = NPH * NH                   # 2016 q rows per core
LQP = 2048                      # padded q rows (16 x 128)
MP = 4096                       # padded contraction dim (16 x 256)
N_CORES = 8
NM = 16                         # 256-row DoubleRow contraction chunks
ND = CKK // 128                 # 6 stationary hf d-tiles of 128
QB = 512                        # attnT columns per band (= max psum free dim)
N_QB = LQP // QB                # 4 q-blocks
SCALE = float(2.0 ** 17)        # attn fp8 pre-scale (max ~137 < 240 = e4m3 max)

LAST_RESULT = None              # BassKernelResults of the most recent run


# ------------------------------------------------------------- host helpers
def _bilinear_up_matrix() -> np.ndarray:
    """U (512, 256): exact jax.image.resize 'bilinear' 256->512 upsample.

    Half-pixel centers: src(o) = o/2 - 0.25; triangle weights, renormalized
    at the edges (matches jax's scale_and_translate for scale 2 upsampling).
    """
    U = np.zeros((HH, HL), np.float32)
    for o in range(HH):
        src = o / 2.0 - 0.25
        i0 = int(np.floor(src))
        f = src - i0
        w = {i0: 1.0 - f, i0 + 1: f}
        valid = {i: wi for i, wi in w.items() if 0 <= i < HL and wi > 0}
        tot = sum(valid.values())
        for i, wi in valid.items():
            U[o, i] = wi / tot
    return U


_U = _bilinear_up_matrix()


def _upsample2(x: np.ndarray) -> np.ndarray:
    """(..., 256, 256) -> (..., 512, 512) bilinear, exact jax semantics."""
    lead = x.shape[:-2]
    xf = x.reshape((-1, HL, HL)).astype(np.float32)
    y = np.einsum("yi,nij,xj->nyx", _U, xf, _U, optimize=True)
    return y.reshape(lead + (HH, HH)).astype(np.float32)


def _unfold_hf(x_hr_b: np.ndarray, blur_hr_b: np.ndarray) -> np.ndarray:
    """hf (L, CKK): unfold(x_hr - blur_hr, k=16, s=8), m=(ph,pw), d=(c,i,j)."""
    d = (x_hr_b - blur_hr_b).astype(np.float32)          # (C, 512, 512)
    win = np.lib.stride_tricks.sliding_window_view(d, (K, K), axis=(1, 2))
    win = win[:, ::S, ::S]                                # (C, 63, 63, 16, 16)
    return np.ascontiguousarray(
        win.transpose(1, 2, 0, 3, 4).reshape(L, CKK))


def _fold(cols: np.ndarray) -> np.ndarray:
    """cols (B, CKK, L) -> overlap-add (B, C, 512, 512) (reference col2im)."""
    c6 = cols.reshape(B, C, K, K, NH, NH)
    out = np.zeros((B, C, HH, HH), np.float32)
    for i in range(K):
        for j in range(K):
            out[:, :, i:i + S * NH:S, j:j + S * NH:S] += c6[:, :, i, j]
    return out


_NORM = None


def _norm_map() -> np.ndarray:
    global _NORM
    if _NORM is None:
        _NORM = _fold(np.ones((B, CKK, L), np.float32))
        _NORM = np.maximum(_NORM, 1e-8)
    return _NORM


def _f8(x: np.ndarray) -> np.ndarray:
    import ml_dtypes
    return x.astype(ml_dtypes.float8_e4m3)


# ------------------------------------------------------------ device kernel
_NC = None


def _build_nc(mode: str = "full"):
    """SPMD bass program: rec = attnT.T @ hf, fp8e4m3 DoubleRow matmuls.

    mode: 'full' (the real kernel), 'dma' (input streaming only) or 'mm'
    (matmul schedule off one resident band) — bisect probes for perf work.
    """
    import bass_rust
    import concourse.bass as bass
    import concourse.mybir as mybir
    from concourse.tile import TileContext
    from concourse.vector_clock import ScopedClock

    # Walrus in this build rejects ctrl instructions carrying >2 sem waits;
    # Tile's exit drain waits on every live semaphore.  Split those waits
    # across single-wait drain instructions.
    def _drain_and_barrier(self, tick_clock, wait_clock):
        nc = self.nc
        drain_inst = nc.sync.drain()
        wait_clock.add_sem_waits(
            drain_inst.ins, ScopedClock({None: tick_clock.global_clock}))
        si = drain_inst.ins.sync_info
        waits = list(si.on_wait) if si is not None else []
        if len(waits) > 1:
            drain_inst.ins.sync_info = bass_rust.SyncInfo(
                on_update=list(si.on_update), on_wait=waits[:1])
            for w in waits[1:]:
                d2 = nc.sync.drain()
                d2.ins.sync_info = bass_rust.SyncInfo(on_update=[], on_wait=[w])
        nc.all_engine_barrier()
        popped = nc._tile_sem_poison_stack.pop()
        assert popped is self._sem_poison
        nc.clear_and_free_semaphores(list(self.sems.allocated().values()))
        nc.all_engine_barrier()

    TileContext._drain_and_barrier = _drain_and_barrier

    # Engine sem-name prefix per engine type, for the self-wait post-pass.
    _ENG_SEM = {
        mybir.EngineType.PE: "PE_",
        mybir.EngineType.DVE: "DVE_",
        mybir.EngineType.Activation: "Activation_",
        mybir.EngineType.SP: "SP_",
        mybir.EngineType.Pool: "Pool_",
    }

    band_nops = []       # per-band SP wait-carrier nops, filled at build time
    prelude_nops = []    # (engine, nop) last-resort wait carriers, per context

    def _split_excess_waits(nc):
        """Walrus in this build caps sem waits per instruction (1 for DMA,
        2 otherwise).  Two legal rewrites bring Tile's output under the cap:
          - drop self-engine waits (WAW on a reused slot): engines complete
            in order, so an earlier same-engine producer is already done;
          - hoist remaining excess waits onto the nearest *preceding*
            same-engine instruction with spare capacity — the sequencer
            executes waits in program order, so waiting earlier is strictly
            more conservative.  (Producers of hoisted waits are tile-slot
            reuses >= one full band older, so no deadlock is possible.)
        """
        import bass_rust as _br

        nop_names = {i.ins.name for i in band_nops}
        prelude_by_name = {i.ins.name: i.ins for _, i in prelude_nops}

        def cap(inst):
            # Empirically this walrus accepts at most ONE sem wait per
            # instruction across every struct we hit (DMA, ACT, LW/matmul,
            # ctrl drain).
            return 1

        def set_waits(inst, waits):
            si = inst.sync_info
            ups = list(si.on_update) if si else []
            inst.sync_info = _br.SyncInfo(on_update=ups, on_wait=waits)

        def merge_wait(inst, w):
            """Add wait w to inst, merging same-sem waits by max value."""
            si = inst.sync_info
            waits = list(si.on_wait) if si else []
            for i, ex in enumerate(waits):
                if ex.ant_name == w.ant_name:
                    if w.wait_value > ex.wait_value:
                        waits[i] = w
                    set_waits(inst, waits)
                    return
            set_waits(inst, waits + [w])

        for bb in nc.main_func.blocks:
            cur_nop = None          # most recent band-carrier nop on SP
            streams = {}            # engine -> prior instructions, in order
            bb_preludes = {}        # engine -> prelude nops IN THIS BB only
            for inst in bb.instructions:
                stream = streams.setdefault(inst.engine, [])
                if inst.name in prelude_by_name:
                    bb_preludes.setdefault(inst.engine, []).append(inst)
                    stream.append(inst)
                    continue
                if inst.name in nop_names:
                    cur_nop = inst
                    stream.append(inst)
                    continue
                si = inst.sync_info
                if si is None:
                    stream.append(inst)
                    continue
                waits = list(si.on_wait)
                if len(waits) <= cap(inst):
                    stream.append(inst)
                    continue
                # 1) drop self-engine waits (in-order engines: an earlier
                #    same-engine producer has completed by issue time)
                pfx = _ENG_SEM.get(inst.engine)
                waits = [w for w in waits
                         if not (pfx and w.ant_name.startswith(pfx))]
                # 1b) a WAR wait on the ACT dummy-read is implied by the WAR
                #     wait on the ACT-issued output DMA (same sequencer,
                #     in-order: dummy completed before the DMA was issued)
                if (len(waits) > cap(inst)
                        and any(w.ant_name.startswith("DMAHW") for w in waits)):
                    waits = [w for w in waits
                             if not w.ant_name.startswith("Activation_")]
                if len(waits) > cap(inst):
                    # keep one wait (prefer the DMA-lane RAW for DMAs), hoist
                    # the rest onto earlier same-engine instructions — waits
                    # execute in sequencer program order, so hoisting is
                    # strictly more conservative.  Producers of hoisted waits
                    # are tile-slot reuses from >= 2 pipeline stages earlier,
                    # so a bounded backward hoist cannot deadlock.
                    if type(inst).__name__ == "InstDMACopy":
                        keep = ([w for w in waits if w.ant_name.startswith("DMAHW")]
                                or waits)[:1]
                    else:
                        keep = waits[:1]
                    hoist = [w for w in waits if w not in keep]
                    for w in hoist:
                        placed = False
                        if inst.engine == mybir.EngineType.SP and cur_nop is not None:
                            merge_wait(cur_nop, w)
                            placed = True
                        else:
                            for prior in reversed(stream[-50:]):
                                psi = prior.sync_info
                                pw = list(psi.on_wait) if psi else []
                                if len(pw) < cap(prior):
                                    set_waits(prior, pw + [w])
                                    placed = True
                                    break
                        if not placed:
                            # last resort: prelude nop on this engine (they
                            # sit at the head of this context's stream)
                            for pn in bb_preludes.get(inst.engine, []):
                                psi = pn.sync_info
                                pw = list(psi.on_wait) if psi else []
                                same = [x for x in pw if x.ant_name == w.ant_name]
                                if same or len(pw) < 1:
                                    merge_wait(pn, w)
                                    placed = True
                                    break
                        assert placed, (
                            f"{inst.name}: no carrier for {w.ant_name}")
                    waits = keep
                assert len(waits) <= cap(inst), (
                    f"{inst.name}: still {len(waits)} waits")
                set_waits(inst, waits)
                stream.append(inst)

    dt = mybir.dt
    f32 = dt.float32
    f8 = dt.float8e4
    bf16 = dt.bfloat16
    DR = mybir.MatmulPerfMode.DoubleRow

    nc = bass.Bass(target_bir_lowering=False)
    # attnT8[qb*128 + p, 2c + i, qcol] = attn_scaled[q0 + qb*512 + qcol,
    #                                                c*256 + i*128 + p]
    attnT = nc.dram_tensor("attnT", [N_QB * 128, 2 * NM, QB], f8,
                           kind="ExternalInput")
    # hf8[p, 2c + i, d] = hf[c*256 + i*128 + p, d]
    hf = nc.dram_tensor("hf", [128, 2 * NM, CKK], f8, kind="ExternalInput")
    # rec[p, qb*ND + dt, qcol] = rec_dq[dt*128 + p, qb*512 + qcol]
    rec = nc.dram_tensor("rec", [128, N_QB * ND, QB], bf16,
                         kind="ExternalOutput")

    # hf lives in a raw SBUF tensor so it survives across the TileContexts
    # (loaded once in a prologue context, read-only afterwards).  hf is the
    # STATIONARY operand: out[d, q] accumulates hf_chunk.T @ attn_chunk, so
    # every matmul runs at the ISA maximum (K=256 DoubleRow, M=128, N=512)
    # and the whole contraction is 4 q-blocks x 6 d-tiles x 16 chunks = 384
    # matmuls.
    hf_raw = nc.alloc_sbuf_tensor("hf_raw", [128, 2 * NM, CKK], f8)
    hf_sb = hf_raw.ap()
    # q-block 0's attn band also loads in the prologue (dedicated raw
    # buffer, concurrent with the hf DMA) so the main context's first
    # matmul group isn't serialized behind BOTH loads.
    att0_raw = nc.alloc_sbuf_tensor("att0_raw", [128, 2 * NM, QB], f8)

    with TileContext(nc):
        nc.sync.dma_start(hf_sb[:, :, :], hf[:, :, :])
        nc.sync.dma_start(att0_raw.ap()[:, :, :], attnT[0:128, :, :])

    with TileContext(nc) as tc:
        with (
            tc.tile_pool(name="attp", bufs=2) as attp,
            tc.tile_pool(name="recp", bufs=3) as recp,
            tc.tile_pool(name="psp", bufs=3, space="PSUM") as psp,
        ):
            for eng_name, eng in (("tensor", nc.tensor),
                                  ("vector", nc.vector),
                                  ("scalar", nc.scalar)):
                for i in range(8):
                    prelude_nops.append(
                        (eng.engine,
                         eng.nop(hint=f"prelude_{eng_name}_{i}")))

            last_a = None
            for qb in range(N_QB):
                # SP wait-carrier: absorbs the attn-load WAR waits
                # (band-2 consumers) so each load keeps only its
                # DMA-lane wait.
                band_nops.append(nc.sync.nop(hint=f"band{qb}_carrier"))
                if qb == 0 or (mode == "mm" and last_a is not None):
                    a = last_a = last_a or att0_raw.ap()
                else:
                    a = attp.tile([128, 2 * NM, QB], f8, tag="at")
                    nc.sync.dma_start(
                        a[:, :, :], attnT[qb * 128:(qb + 1) * 128, :, :])
                    last_a = a
                if mode == "dma":
                    continue
                ro = recp.tile([128, ND, QB], bf16, tag="ro")
                for dtp in range(ND // 2):
                    # one 2-bank psum tile per d-tile pair
                    p = psp.tile([128, 2, QB], f32, tag="p")
                    for dt2 in range(2):
                        d0 = (2 * dtp + dt2) * 128
                        for c in range(NM):
                            nc.tensor.matmul(
                                p[:, dt2:dt2 + 1, :],
                                hf_sb[:, 2 * c:2 * c + 2, d0:d0 + 128],
                                a[:, 2 * c:2 * c + 2, :],
                                start=(c == 0), stop=(c == NM - 1),
                                perf_mode=DR)
                    # Each copyback needs a RAW (PE) and a WAR (output DMA)
                    # wait but the ISA takes one per instruction.  The tiny
                    # psum read carries the PE wait (pinned in the schedule
                    # by its RAW dep), so Tile elides the PE wait from the
                    # big cast-copy, which keeps only the WAR wait.
                    tny = recp.tile([128, 1, 1], f32, tag=f"tny{dtp}")
                    nc.vector.tensor_copy(tny[:, :, :], p[:, 0:1, 0:1])
                    nc.vector.tensor_copy(
                        ro[:, 2 * dtp:2 * dtp + 2, :], p[:, :, :])
                # ACT observes the DVE copies via this cheap read (of the
                # LAST copy's output), so the ACT-issued output DMA needs no
                # extra DVE wait of its own (Tile elides observed ticks).
                dmy = recp.tile([128, 1, 1], bf16, tag="dmy")
                nc.scalar.copy(dmy[:, :, :], ro[:, ND - 1:ND, QB - 1:QB])
                nc.scalar.dma_start(
                    rec[:, qb * ND:(qb + 1) * ND, :], ro[:, :, :])
    _split_excess_waits(nc)
    return nc


def _get_nc():
    global _NC
    if _NC is None:
        _NC = _build_nc()
    return _NC


# ---------------------------------------------------------------- benchmark
def bench(in_maps, iters: int = 10):
    """Steady-state per-execution time of the compiled NEFF.

    Re-implements bass2jax.run_bass_via_pjrt's jit/shard_map wrapping, but
    device_puts the inputs once, pre-creates donated output buffers ON
    DEVICE (no host->device zero traffic in the timed region), dispatches
    two batches of executions asynchronously and reports the MARGINAL time
    per execution between them — the fixed dispatch/sync overhead of the
    axon client (~115 ms per batch, measured with an empty NEFF) cancels
    out, leaving the steady-state pipelined per-NEFF device time.
    """
    import time

    import jax
    import jax.numpy as jnp
    import numpy as np
    from jax.experimental.shard_map import shard_map
    from jax.sharding import Mesh, NamedSharding, PartitionSpec

    import concourse.bass2jax as bass2jax
    import concourse.mybir as mybir

    nc = _get_nc()
    bass2jax.install_neuronx_cc_hook()

    in_names, out_names, out_avals, zero_outs = [], [], [], []
    for alloc in nc.m.functions[0].allocations:
        if not isinstance(alloc, mybir.MemoryLocationSet):
            continue
        name = alloc.memorylocations[0].name
        if alloc.kind == "ExternalInput":
            in_names.append(name)
        elif alloc.kind == "ExternalOutput":
            shape = tuple(alloc.tensor_shape)
            dtype = mybir.dt.np(alloc.dtype)
            out_names.append(name)
            out_avals.append(jax.core.ShapedArray(shape, dtype))
            zero_outs.append(np.zeros(shape, dtype))
    n_params = len(in_names)
    n_outs = len(out_avals)
    all_names = in_names + out_names
    donate = tuple(range(n_params, n_params + n_outs))

    def _body(*args):
        outs = bass2jax._bass_exec_p.bind(
            *args,
            out_avals=tuple(out_avals),
            in_names=tuple(all_names),
            out_names=tuple(out_names),
            lowering_input_output_aliases=(),
            sim_require_finite=True,
            sim_require_nnan=True,
            nc=nc,
        )
        return tuple(outs)

    devices = jax.devices()[:N_CORES]
    mesh = Mesh(np.asarray(devices), ("core",))
    sh = NamedSharding(mesh, PartitionSpec("core"))
    sharded = jax.jit(
        shard_map(_body, mesh=mesh,
                  in_specs=(PartitionSpec("core"),) * (n_params + n_outs),
                  out_specs=(PartitionSpec("core"),) * n_outs,
                  check_rep=False),
        donate_argnums=donate, keep_unused=True)

    concat_in = []
    for nm in in_names:
        if nm == "partition_id":
            concat_in.append(
                np.arange(N_CORES, dtype=np.uint32).reshape(N_CORES, 1))
        else:
            concat_in.append(np.concatenate(
                [np.asarray(in_maps[c][nm]) for c in range(N_CORES)], 0))
    dev_in = [jax.device_put(a, sh) for a in concat_in]
    # Donated output buffers created on device — no host->device transfer.
    zeros_fn = jax.jit(
        lambda: tuple(
            jnp.zeros((N_CORES * z.shape[0], *z.shape[1:]), z.dtype)
            for z in zero_outs),
        out_shardings=tuple(sh for _ in zero_outs))

    warm = sharded(*dev_in, *zeros_fn())
    jax.block_until_ready(warm)

    def timed(n):
        zbufs = [zeros_fn() for _ in range(n)]
        jax.block_until_ready(zbufs)
        t0 = time.perf_counter()
        outs = [sharded(*dev_in, *zbufs[i]) for i in range(n)]
        jax.block_until_ready(outs)
        t1 = time.perf_counter()
        del outs
        return t1 - t0

    # Median of repeated two-point marginal estimates: robust both to the
    # fixed per-batch dispatch overhead (cancels in the difference) and to
    # the remote terminal's load fluctuations (median over reps).
    n1, n2 = iters, iters + 40
    margs = []
    for _ in range(5):
        t1 = timed(n1)
        t2 = timed(n2)
        margs.append((t2 - t1) / (n2 - n1))
    per_call_ns = sorted(margs)[len(margs) // 2] * 1e9
    return per_call_ns, warm


# ------------------------------------------------------------------- kernel
def _prepare(x_hr, x_lr_inpainted, attn_map, x_lr_blurred):
    """Host sharding prep: upsample, unfold, quantize, per-core shards."""
    x_hr = np.asarray(x_hr, np.float32)
    x_lr_inpainted = np.asarray(x_lr_inpainted, np.float32)
    attn_map = np.asarray(attn_map, np.float32)
    x_lr_blurred = np.asarray(x_lr_blurred, np.float32)

    blur_hr = _upsample2(x_lr_blurred)                    # (B, C, 512, 512)
    base = _upsample2(x_lr_inpainted)                     # (B, C, 512, 512)

    q_starts = (0, L - LQ)                                # 0 and 1953
    in_maps = []
    hf_pack_cache = {}
    for core in range(N_CORES):
        b, half = core // 2, core % 2
        if b not in hf_pack_cache:
            hfp = np.zeros((MP, CKK), np.float32)
            hfp[:L] = _unfold_hf(x_hr[b], blur_hr[b])
            # [m, d] -> [p, 2c+i, d], m = c*256 + i*128 + p
            hf8 = _f8(hfp).reshape(NM, 2, 128, CKK).transpose(2, 0, 1, 3)
            hf_pack_cache[b] = np.ascontiguousarray(
                hf8.reshape(128, 2 * NM, CKK))
        q0 = q_starts[half]
        atq = np.zeros((MP, LQP), _f8(np.zeros(1)).dtype)
        atq[:L, :LQ] = _f8(attn_map[b, 0, q0:q0 + LQ, :] * SCALE).T
        # [m, q] -> [qb*128 + p, 2c+i, qcol], m = c*256+i*128+p, q = qb*512+qcol
        at = atq.reshape(NM, 2, 128, N_QB, QB).transpose(3, 2, 0, 1, 4)
        at = np.ascontiguousarray(at.reshape(N_QB * 128, 2 * NM, QB))
        in_maps.append({"attnT": at, "hf": hf_pack_cache[b]})
    return in_maps, base


def _rec_dq(rec_core: np.ndarray) -> np.ndarray:
    """Device rec [128, N_QB*ND, 512] bf16 -> [768 d, 2048 q] fp32."""
    r = np.asarray(rec_core, np.float32).reshape(128, N_QB, ND, QB)
    return r.transpose(2, 0, 1, 3).reshape(CKK, LQP)


def _finish(per_core_rec, base):
    """Gather: stitch q-halves, fold, de-scale + normalize, add base."""
    cols = np.empty((B, CKK, L), np.float32)
    for b in range(B):
        rec_a = _rec_dq(per_core_rec[2 * b])                   # (768, 2048)
        rec_b = _rec_dq(per_core_rec[2 * b + 1])
        cols[b, :, :LQ] = rec_a[:, :LQ]
        cols[b, :, LQ:] = rec_b[:, 2 * LQ - L:LQ]
    img = _fold(cols)
    out = base + img / (_norm_map() * SCALE)
    return out.astype(np.float32)


def kernel(x_hr, x_lr_inpainted, attn_map, x_lr_blurred):
    global LAST_RESULT
    from concourse.bass_utils import run_bass_kernel_spmd

    in_maps, base = _prepare(x_hr, x_lr_inpainted, attn_map, x_lr_blurred)
    nc = _get_nc()
    trace = bool(os.environ.get("KERNEL_TRACE"))
    res = run_bass_kernel_spmd(nc, in_maps, list(range(N_CORES)), trace=trace)
    LAST_RESULT = res
    return _finish([res.results[c]["rec"] for c in range(N_CORES)], base)


# revision 17
# speedup vs baseline: 1.0719x; 1.0719x over previous
"""AttentionUpscaling Trainium2 kernel.

Strategy (8 NeuronCores):
  - Pure data parallelism over batch (4) x query-half (2): each core owns one
    (batch, q-half) shard of the L x L attention matmul (the ~97 GFLOP that
    dominate this problem).
  - Host side (sharding prep): bilinear 2x upsample (exact jax semantics via a
    sparse banded matrix), unfold of the high-frequency residual, fp8e4m3
    quantization (attn pre-scaled by 2^17 into the fp8 sweet spot), and
    device-layout packing so every band is ONE contiguous-per-partition DMA.
  - Device side (SPMD bass/Tile program, same NEFF on all 8 cores):
    rec[q, d] = sum_m attnT[m, q] * hf[m, d]; contraction over m=4096 in 16
    DoubleRow chunks of 256 (fp8e4m3 at 2x PE rate), hf SBUF-resident (one
    3 MB DMA), attnT streamed in 256-column bands (one 1 MB DMA per band,
    double-buffered), PSUM accumulation, DVE copyback with fused fp32->bf16
    cast, HWDGE DMA out (bf16 halves the output traffic).
  - Host side (gather): overlap-add fold + overlap-count normalization (the
    2^17 attn pre-scale is folded into the normalization divide) + base image
    add, then stitch the two q-halves per batch.
"""

import os

import numpy as np

# ---------------------------------------------------------------- constants
B, C = 4, 3
HH = 512          # HR height/width
HL = 256          # LR height/width
K = 16            # HR patch size
S = 8             # HR stride
NH = (HH - K) // S + 1          # 63 patches per axis
L = NH * NH                     # 3969 patches
CKK = C * K * K                 # 768
NPH = 32                        # patch-rows per core (ph 0..31 / 31..62)
LQ = NPH * NH                   # 2016 q rows per core
LQP = 2048                      # padded q rows (16 x 128)
MP = 4096                       # padded contraction dim (16 x 256)
N_CORES = 8
NM = 16                         # 256-row DoubleRow contraction chunks
ND = CKK // 128                 # 6 stationary hf d-tiles of 128
QB = 512                        # attnT columns per band (= max psum free dim)
N_QB = LQP // QB                # 4 q-blocks
SCALE = float(2.0 ** 17)        # attn fp8 pre-scale (max ~137 < 240 = e4m3 max)

LAST_RESULT = None              # BassKernelResults of the most recent run


# ------------------------------------------------------------- host helpers
def _bilinear_up_matrix() -> np.ndarray:
    """U (512, 256): exact jax.image.resize 'bilinear' 256->512 upsample.

    Half-pixel centers: src(o) = o/2 - 0.25; triangle weights, renormalized
    at the edges (matches jax's scale_and_translate for scale 2 upsampling).
    """
    U = np.zeros((HH, HL), np.float32)
    for o in range(HH):
        src = o / 2.0 - 0.25
        i0 = int(np.floor(src))
        f = src - i0
        w = {i0: 1.0 - f, i0 + 1: f}
        valid = {i: wi for i, wi in w.items() if 0 <= i < HL and wi > 0}
        tot = sum(valid.values())
        for i, wi in valid.items():
            U[o, i] = wi / tot
    return U


_U = _bilinear_up_matrix()


def _upsample2(x: np.ndarray) -> np.ndarray:
    """(..., 256, 256) -> (..., 512, 512) bilinear, exact jax semantics."""
    lead = x.shape[:-2]
    xf = x.reshape((-1, HL, HL)).astype(np.float32)
    y = np.einsum("yi,nij,xj->nyx", _U, xf, _U, optimize=True)
    return y.reshape(lead + (HH, HH)).astype(np.float32)


def _unfold_hf(x_hr_b: np.ndarray, blur_hr_b: np.ndarray) -> np.ndarray:
    """hf (L, CKK): unfold(x_hr - blur_hr, k=16, s=8), m=(ph,pw), d=(c,i,j)."""
    d = (x_hr_b - blur_hr_b).astype(np.float32)          # (C, 512, 512)
    win = np.lib.stride_tricks.sliding_window_view(d, (K, K), axis=(1, 2))
    win = win[:, ::S, ::S]                                # (C, 63, 63, 16, 16)
    return np.ascontiguousarray(
        win.transpose(1, 2, 0, 3, 4).reshape(L, CKK))


def _fold(cols: np.ndarray) -> np.ndarray:
    """cols (B, CKK, L) -> overlap-add (B, C, 512, 512) (reference col2im)."""
    c6 = cols.reshape(B, C, K, K, NH, NH)
    out = np.zeros((B, C, HH, HH), np.float32)
    for i in range(K):
        for j in range(K):
            out[:, :, i:i + S * NH:S, j:j + S * NH:S] += c6[:, :, i, j]
    return out


_NORM = None


def _norm_map() -> np.ndarray:
    global _NORM
    if _NORM is None:
        _NORM = _fold(np.ones((B, CKK, L), np.float32))
        _NORM = np.maximum(_NORM, 1e-8)
    return _NORM


def _f8(x: np.ndarray) -> np.ndarray:
    import ml_dtypes
    return x.astype(ml_dtypes.float8_e4m3)


# ------------------------------------------------------------ device kernel
_NC = None


def _build_nc(mode: str = "full"):
    """SPMD bass program: rec = attnT.T @ hf, fp8e4m3 DoubleRow matmuls.

    mode: 'full' (the real kernel), 'dma' (input streaming only) or 'mm'
    (matmul schedule off one resident band) — bisect probes for perf work.
    """
    import bass_rust
    import concourse.bass as bass
    import concourse.mybir as mybir
    from concourse.tile import TileContext
    from concourse.vector_clock import ScopedClock

    # Walrus in this build rejects ctrl instructions carrying >2 sem waits;
    # Tile's exit drain waits on every live semaphore.  Split those waits
    # across single-wait drain instructions.
    def _drain_and_barrier(self, tick_clock, wait_clock):
        nc = self.nc
        drain_inst = nc.sync.drain()
        wait_clock.add_sem_waits(
            drain_inst.ins, ScopedClock({None: tick_clock.global_clock}))
        si = drain_inst.ins.sync_info
        waits = list(si.on_wait) if si is not None else []
        if len(waits) > 1:
            drain_inst.ins.sync_info = bass_rust.SyncInfo(
                on_update=list(si.on_update), on_wait=waits[:1])
            for w in waits[1:]:
                d2 = nc.sync.drain()
                d2.ins.sync_info = bass_rust.SyncInfo(on_update=[], on_wait=[w])
        nc.all_engine_barrier()
        popped = nc._tile_sem_poison_stack.pop()
        assert popped is self._sem_poison
        nc.clear_and_free_semaphores(list(self.sems.allocated().values()))
        nc.all_engine_barrier()

    TileContext._drain_and_barrier = _drain_and_barrier

    # Engine sem-name prefix per engine type, for the self-wait post-pass.
    _ENG_SEM = {
        mybir.EngineType.PE: "PE_",
        mybir.EngineType.DVE: "DVE_",
        mybir.EngineType.Activation: "Activation_",
        mybir.EngineType.SP: "SP_",
        mybir.EngineType.Pool: "Pool_",
    }

    band_nops = []       # per-band SP wait-carrier nops, filled at build time
    prelude_nops = []    # (engine, nop) last-resort wait carriers, per context

    def _split_excess_waits(nc):
        """Walrus in this build caps sem waits per instruction (1 for DMA,
        2 otherwise).  Two legal rewrites bring Tile's output under the cap:
          - drop self-engine waits (WAW on a reused slot): engines complete
            in order, so an earlier same-engine producer is already done;
          - hoist remaining excess waits onto the nearest *preceding*
            same-engine instruction with spare capacity — the sequencer
            executes waits in program order, so waiting earlier is strictly
            more conservative.  (Producers of hoisted waits are tile-slot
            reuses >= one full band older, so no deadlock is possible.)
        """
        import bass_rust as _br

        nop_names = {i.ins.name for i in band_nops}
        prelude_by_name = {i.ins.name: i.ins for _, i in prelude_nops}

        def cap(inst):
            # Empirically this walrus accepts at most ONE sem wait per
            # instruction across every struct we hit (DMA, ACT, LW/matmul,
            # ctrl drain).
            return 1

        def set_waits(inst, waits):
            si = inst.sync_info
            ups = list(si.on_update) if si else []
            inst.sync_info = _br.SyncInfo(on_update=ups, on_wait=waits)

        def merge_wait(inst, w):
            """Add wait w to inst, merging same-sem waits by max value."""
            si = inst.sync_info
            waits = list(si.on_wait) if si else []
            for i, ex in enumerate(waits):
                if ex.ant_name == w.ant_name:
                    if w.wait_value > ex.wait_value:
                        waits[i] = w
                    set_waits(inst, waits)
                    return
            set_waits(inst, waits + [w])

        for bb in nc.main_func.blocks:
            cur_nop = None          # most recent band-carrier nop on SP
            streams = {}            # engine -> prior instructions, in order
            bb_preludes = {}        # engine -> prelude nops IN THIS BB only
            for inst in bb.instructions:
                stream = streams.setdefault(inst.engine, [])
                if inst.name in prelude_by_name:
                    bb_preludes.setdefault(inst.engine, []).append(inst)
                    stream.append(inst)
                    continue
                if inst.name in nop_names:
                    cur_nop = inst
                    stream.append(inst)
                    continue
                si = inst.sync_info
                if si is None:
                    stream.append(inst)
                    continue
                waits = list(si.on_wait)
                if len(waits) <= cap(inst):
                    stream.append(inst)
                    continue
                # 1) drop self-engine waits (in-order engines: an earlier
                #    same-engine producer has completed by issue time)
                pfx = _ENG_SEM.get(inst.engine)
                waits = [w for w in waits
                         if not (pfx and w.ant_name.startswith(pfx))]
                # 1b) a WAR wait on the ACT dummy-read is implied by the WAR
                #     wait on the ACT-issued output DMA (same sequencer,
                #     in-order: dummy completed before the DMA was issued)
                if (len(waits) > cap(inst)
                        and any(w.ant_name.startswith("DMAHW") for w in waits)):
                    waits = [w for w in waits
                             if not w.ant_name.startswith("Activation_")]
                if len(waits) > cap(inst):
                    # keep one wait (prefer the DMA-lane RAW for DMAs), hoist
                    # the rest onto earlier same-engine instructions — waits
                    # execute in sequencer program order, so hoisting is
                    # strictly more conservative.  Producers of hoisted waits
                    # are tile-slot reuses from >= 2 pipeline stages earlier,
                    # so a bounded backward hoist cannot deadlock.
                    if type(inst).__name__ == "InstDMACopy":
                        keep = ([w for w in waits if w.ant_name.startswith("DMAHW")]
                                or waits)[:1]
                    else:
                        keep = waits[:1]
                    hoist = [w for w in waits if w not in keep]
                    for w in hoist:
                        placed = False
                        if inst.engine == mybir.EngineType.SP and cur_nop is not None:
                            merge_wait(cur_nop, w)
                            placed = True
                        else:
                            for prior in reversed(stream[-50:]):
                                psi = prior.sync_info
                                pw = list(psi.on_wait) if psi else []
                                if len(pw) < cap(prior):
                                    set_waits(prior, pw + [w])
                                    placed = True
                                    break
                        if not placed:
                            # last resort: prelude nop on this engine (they
                            # sit at the head of this context's stream)
                            for pn in bb_preludes.get(inst.engine, []):
                                psi = pn.sync_info
                                pw = list(psi.on_wait) if psi else []
                                same = [x for x in pw if x.ant_name == w.ant_name]
                                if same or len(pw) < 1:
                                    merge_wait(pn, w)
                                    placed = True
                                    break
                        assert placed, (
                            f"{inst.name}: no carrier for {w.ant_name}")
                    waits = keep
                assert len(waits) <= cap(inst), (
                    f"{inst.name}: still {len(waits)} waits")
                set_waits(inst, waits)
                stream.append(inst)

    dt = mybir.dt
    f32 = dt.float32
    f8 = dt.float8e4
    bf16 = dt.bfloat16
    DR = mybir.MatmulPerfMode.DoubleRow

    nc = bass.Bass(target_bir_lowering=False)
    # attnT8[qb*128 + p, 2c + i, qcol] = attn_scaled[q0 + qb*512 + qcol,
    #                                                c*256 + i*128 + p]
    attnT = nc.dram_tensor("attnT", [N_QB * 128, 2 * NM, QB], f8,
                           kind="ExternalInput")
    # hf8[p, 2c + i, d] = hf[c*256 + i*128 + p, d]
    hf = nc.dram_tensor("hf", [128, 2 * NM, CKK], f8, kind="ExternalInput")
    # rec[p, qb*ND + dt, qcol] = rec_dq[dt*128 + p, qb*512 + qcol]
    rec = nc.dram_tensor("rec", [128, N_QB * ND, QB], bf16,
                         kind="ExternalOutput")
    # Version tag: kernel revisions keep identical I/O signatures, which
    # lets a stale NEFF survive HLO-keyed compile caches.  A [1, VER]
    # input (DMA'd so it isn't pruned) bumps the signature per revision.
    VER = 2
    cfg = nc.dram_tensor("cfg", [1, VER], mybir.dt.uint32,
                         kind="ExternalInput")
    cfg_raw = nc.alloc_sbuf_tensor("cfg_raw", [1, VER], mybir.dt.uint32)

    # hf lives in a raw SBUF tensor so it survives across the TileContexts
    # (loaded once in a prologue context, read-only afterwards).  hf is the
    # STATIONARY operand: out[d, q] accumulates hf_chunk.T @ attn_chunk, so
    # every matmul runs at the ISA maximum (K=256 DoubleRow, M=128, N=512)
    # and the whole contraction is 4 q-blocks x 6 d-tiles x 16 chunks = 384
    # matmuls.
    hf_raw = nc.alloc_sbuf_tensor("hf_raw", [128, 2 * NM, CKK], f8)
    hf_sb = hf_raw.ap()
    # q-block 0's attn band also loads in the prologue (dedicated raw
    # buffer, concurrent with the hf DMA) so the main context's first
    # matmul group isn't serialized behind BOTH loads.
    att0_raw = nc.alloc_sbuf_tensor("att0_raw", [128, 2 * NM, QB], f8)

    with TileContext(nc):
        nc.sync.dma_start(hf_sb[:, :, :], hf[:, :, :])
        nc.sync.dma_start(att0_raw.ap()[:, :, :], attnT[0:128, :, :])
        nc.sync.dma_start(cfg_raw.ap()[:, :], cfg[:, :])

    with TileContext(nc) as tc:
        with (
            tc.tile_pool(name="attp", bufs=2) as attp,
            tc.tile_pool(name="recp", bufs=3) as recp,
            tc.tile_pool(name="psp", bufs=3, space="PSUM") as psp,
        ):
            for eng_name, eng in (("tensor", nc.tensor),
                                  ("vector", nc.vector),
                                  ("scalar", nc.scalar)):
                for i in range(8):
                    prelude_nops.append(
                        (eng.engine,
                         eng.nop(hint=f"prelude_{eng_name}_{i}")))

            last_a = None
            for qb in range(N_QB):
                # SP wait-carrier: absorbs the attn-load WAR waits
                # (band-2 consumers) so each load keeps only its
                # DMA-lane wait.
                band_nops.append(nc.sync.nop(hint=f"band{qb}_carrier"))
                if qb == 0 or (mode == "mm" and last_a is not None):
                    a = last_a = last_a or att0_raw.ap()
                else:
                    a = attp.tile([128, 2 * NM, QB], f8, tag="at")
                    nc.sync.dma_start(
                        a[:, :, :], attnT[qb * 128:(qb + 1) * 128, :, :])
                    last_a = a
                if mode == "dma":
                    continue
                ro = recp.tile([128, ND, QB], bf16, tag="ro")
                for dtp in range(ND // 2):
                    # one 2-bank psum tile per d-tile pair
                    p = psp.tile([128, 2, QB], f32, tag="p")
                    for dt2 in range(2):
                        d0 = (2 * dtp + dt2) * 128
                        for c in range(NM):
                            nc.tensor.matmul(
                                p[:, dt2:dt2 + 1, :],
                                hf_sb[:, 2 * c:2 * c + 2, d0:d0 + 128],
                                a[:, 2 * c:2 * c + 2, :],
                                start=(c == 0), stop=(c == NM - 1),
                                perf_mode=DR)
                    # Each copyback needs a RAW (PE) and a WAR (output DMA)
                    # wait but the ISA takes one per instruction.  The tiny
                    # psum read carries the PE wait (pinned in the schedule
                    # by its RAW dep), so Tile elides the PE wait from the
                    # big cast-copy, which keeps only the WAR wait.
                    tny = recp.tile([128, 1, 1], f32, tag=f"tny{dtp}")
                    nc.vector.tensor_copy(tny[:, :, :], p[:, 0:1, 0:1])
                    nc.vector.tensor_copy(
                        ro[:, 2 * dtp:2 * dtp + 2, :], p[:, :, :])
                # ACT observes the DVE copies via this cheap read (of the
                # LAST copy's output), so the ACT-issued output DMA needs no
                # extra DVE wait of its own (Tile elides observed ticks).
                dmy = recp.tile([128, 1, 1], bf16, tag="dmy")
                nc.scalar.copy(dmy[:, :, :], ro[:, ND - 1:ND, QB - 1:QB])
                nc.scalar.dma_start(
                    rec[:, qb * ND:(qb + 1) * ND, :], ro[:, :, :])
    _split_excess_waits(nc)
    return nc


def _get_nc():
    global _NC
    if _NC is None:
        _NC = _build_nc()
    return _NC


# ---------------------------------------------------------------- benchmark
def bench(in_maps, iters: int = 10):
    """Steady-state per-execution time of the compiled NEFF.

    Re-implements bass2jax.run_bass_via_pjrt's jit/shard_map wrapping, but
    device_puts the inputs once, pre-creates donated output buffers ON
    DEVICE (no host->device zero traffic in the timed region), dispatches
    two batches of executions asynchronously and reports the MARGINAL time
    per execution between them — the fixed dispatch/sync overhead of the
    axon client (~115 ms per batch, measured with an empty NEFF) cancels
    out, leaving the steady-state pipelined per-NEFF device time.
    """
    import time

    import jax
    import jax.numpy as jnp
    import numpy as np
    from jax.experimental.shard_map import shard_map
    from jax.sharding import Mesh, NamedSharding, PartitionSpec

    import concourse.bass2jax as bass2jax
    import concourse.mybir as mybir

    nc = _get_nc()
    bass2jax.install_neuronx_cc_hook()

    in_names, out_names, out_avals, zero_outs = [], [], [], []
    for alloc in nc.m.functions[0].allocations:
        if not isinstance(alloc, mybir.MemoryLocationSet):
            continue
        name = alloc.memorylocations[0].name
        if alloc.kind == "ExternalInput":
            in_names.append(name)
        elif alloc.kind == "ExternalOutput":
            shape = tuple(alloc.tensor_shape)
            dtype = mybir.dt.np(alloc.dtype)
            out_names.append(name)
            out_avals.append(jax.core.ShapedArray(shape, dtype))
            zero_outs.append(np.zeros(shape, dtype))
    n_params = len(in_names)
    n_outs = len(out_avals)
    all_names = in_names + out_names
    donate = tuple(range(n_params, n_params + n_outs))

    def _body(*args):
        outs = bass2jax._bass_exec_p.bind(
            *args,
            out_avals=tuple(out_avals),
            in_names=tuple(all_names),
            out_names=tuple(out_names),
            lowering_input_output_aliases=(),
            sim_require_finite=True,
            sim_require_nnan=True,
            nc=nc,
        )
        return tuple(outs)

    devices = jax.devices()[:N_CORES]
    mesh = Mesh(np.asarray(devices), ("core",))
    sh = NamedSharding(mesh, PartitionSpec("core"))
    sharded = jax.jit(
        shard_map(_body, mesh=mesh,
                  in_specs=(PartitionSpec("core"),) * (n_params + n_outs),
                  out_specs=(PartitionSpec("core"),) * n_outs,
                  check_rep=False),
        donate_argnums=donate, keep_unused=True)

    concat_in = []
    for nm in in_names:
        if nm == "partition_id":
            concat_in.append(
                np.arange(N_CORES, dtype=np.uint32).reshape(N_CORES, 1))
        else:
            concat_in.append(np.concatenate(
                [np.asarray(in_maps[c][nm]) for c in range(N_CORES)], 0))
    dev_in = [jax.device_put(a, sh) for a in concat_in]
    # Donated output buffers created on device — no host->device transfer.
    zeros_fn = jax.jit(
        lambda: tuple(
            jnp.zeros((N_CORES * z.shape[0], *z.shape[1:]), z.dtype)
            for z in zero_outs),
        out_shardings=tuple(sh for _ in zero_outs))

    warm = sharded(*dev_in, *zeros_fn())
    jax.block_until_ready(warm)

    def timed(n):
        zbufs = [zeros_fn() for _ in range(n)]
        jax.block_until_ready(zbufs)
        t0 = time.perf_counter()
        outs = [sharded(*dev_in, *zbufs[i]) for i in range(n)]
        jax.block_until_ready(outs)
        t1 = time.perf_counter()
        del outs
        return t1 - t0

    # Median of repeated two-point marginal estimates: robust both to the
    # fixed per-batch dispatch overhead (cancels in the difference) and to
    # the remote terminal's load fluctuations (median over reps).
    n1, n2 = iters, iters + 40
    margs = []
    for _ in range(5):
        t1 = timed(n1)
        t2 = timed(n2)
        margs.append((t2 - t1) / (n2 - n1))
    per_call_ns = sorted(margs)[len(margs) // 2] * 1e9
    return per_call_ns, warm


# ------------------------------------------------------------------- kernel
def _prepare(x_hr, x_lr_inpainted, attn_map, x_lr_blurred):
    """Host sharding prep: upsample, unfold, quantize, per-core shards."""
    x_hr = np.asarray(x_hr, np.float32)
    x_lr_inpainted = np.asarray(x_lr_inpainted, np.float32)
    attn_map = np.asarray(attn_map, np.float32)
    x_lr_blurred = np.asarray(x_lr_blurred, np.float32)

    blur_hr = _upsample2(x_lr_blurred)                    # (B, C, 512, 512)
    base = _upsample2(x_lr_inpainted)                     # (B, C, 512, 512)

    q_starts = (0, L - LQ)                                # 0 and 1953
    in_maps = []
    hf_pack_cache = {}
    for core in range(N_CORES):
        b, half = core // 2, core % 2
        if b not in hf_pack_cache:
            hfp = np.zeros((MP, CKK), np.float32)
            hfp[:L] = _unfold_hf(x_hr[b], blur_hr[b])
            # [m, d] -> [p, 2c+i, d], m = c*256 + i*128 + p
            hf8 = _f8(hfp).reshape(NM, 2, 128, CKK).transpose(2, 0, 1, 3)
            hf_pack_cache[b] = np.ascontiguousarray(
                hf8.reshape(128, 2 * NM, CKK))
        q0 = q_starts[half]
        atq = np.zeros((MP, LQP), _f8(np.zeros(1)).dtype)
        atq[:L, :LQ] = _f8(attn_map[b, 0, q0:q0 + LQ, :] * SCALE).T
        # [m, q] -> [qb*128 + p, 2c+i, qcol], m = c*256+i*128+p, q = qb*512+qcol
        at = atq.reshape(NM, 2, 128, N_QB, QB).transpose(3, 2, 0, 1, 4)
        at = np.ascontiguousarray(at.reshape(N_QB * 128, 2 * NM, QB))
        in_maps.append({"attnT": at, "hf": hf_pack_cache[b],
                        "cfg": np.zeros((1, 2), np.uint32)})
    return in_maps, base


def _rec_dq(rec_core: np.ndarray) -> np.ndarray:
    """Device rec [128, N_QB*ND, 512] bf16 -> [768 d, 2048 q] fp32."""
    r = np.asarray(rec_core, np.float32).reshape(128, N_QB, ND, QB)
    return r.transpose(2, 0, 1, 3).reshape(CKK, LQP)


def _finish(per_core_rec, base):
    """Gather: stitch q-halves, fold, de-scale + normalize, add base."""
    cols = np.empty((B, CKK, L), np.float32)
    for b in range(B):
        rec_a = _rec_dq(per_core_rec[2 * b])                   # (768, 2048)
        rec_b = _rec_dq(per_core_rec[2 * b + 1])
        cols[b, :, :LQ] = rec_a[:, :LQ]
        cols[b, :, LQ:] = rec_b[:, 2 * LQ - L:LQ]
    img = _fold(cols)
    out = base + img / (_norm_map() * SCALE)
    return out.astype(np.float32)


def kernel(x_hr, x_lr_inpainted, attn_map, x_lr_blurred):
    global LAST_RESULT
    from concourse.bass_utils import run_bass_kernel_spmd

    in_maps, base = _prepare(x_hr, x_lr_inpainted, attn_map, x_lr_blurred)
    nc = _get_nc()
    trace = bool(os.environ.get("KERNEL_TRACE"))
    res = run_bass_kernel_spmd(nc, in_maps, list(range(N_CORES)), trace=trace)
    LAST_RESULT = res
    return _finish([res.results[c]["rec"] for c in range(N_CORES)], base)


# revision 18
# speedup vs baseline: 1.3146x; 1.2264x over previous
"""AttentionUpscaling Trainium2 kernel.

Strategy (8 NeuronCores):
  - Pure data parallelism over batch (4) x query-half (2): each core owns one
    (batch, q-half) shard of the L x L attention matmul (the ~97 GFLOP that
    dominate this problem).
  - Host side (sharding prep): bilinear 2x upsample (exact jax semantics via a
    sparse banded matrix), unfold of the high-frequency residual, fp8e4m3
    quantization (attn pre-scaled by 2^17 into the fp8 sweet spot), and
    device-layout packing so every band is ONE contiguous-per-partition DMA.
  - Device side (SPMD bass/Tile program, same NEFF on all 8 cores):
    rec[q, d] = sum_m attnT[m, q] * hf[m, d]; contraction over m=4096 in 16
    DoubleRow chunks of 256 (fp8e4m3 at 2x PE rate), hf SBUF-resident (one
    3 MB DMA), attnT streamed in 256-column bands (one 1 MB DMA per band,
    double-buffered), PSUM accumulation, DVE copyback with fused fp32->bf16
    cast, HWDGE DMA out (bf16 halves the output traffic).
  - Host side (gather): overlap-add fold + overlap-count normalization (the
    2^17 attn pre-scale is folded into the normalization divide) + base image
    add, then stitch the two q-halves per batch.
"""

import os

import numpy as np

# ---------------------------------------------------------------- constants
B, C = 4, 3
HH = 512          # HR height/width
HL = 256          # LR height/width
K = 16            # HR patch size
S = 8             # HR stride
NH = (HH - K) // S + 1          # 63 patches per axis
L = NH * NH                     # 3969 patches
CKK = C * K * K                 # 768
NPH = 32                        # patch-rows per core (ph 0..31 / 31..62)
LQ = NPH * NH                   # 2016 q rows per core
LQP = 2048                      # padded q rows (16 x 128)
MP = 4096                       # padded contraction dim (16 x 256)
N_CORES = 8
NM = 16                         # 256-row DoubleRow contraction chunks
ND = CKK // 128                 # 6 stationary hf d-tiles of 128
QB = 512                        # attnT columns per band (= max psum free dim)
N_QB = LQP // QB                # 4 q-blocks
SCALE = float(2.0 ** 17)        # attn fp8 pre-scale (max ~137 < 240 = e4m3 max)

LAST_RESULT = None              # BassKernelResults of the most recent run


# ------------------------------------------------------------- host helpers
def _bilinear_up_matrix() -> np.ndarray:
    """U (512, 256): exact jax.image.resize 'bilinear' 256->512 upsample.

    Half-pixel centers: src(o) = o/2 - 0.25; triangle weights, renormalized
    at the edges (matches jax's scale_and_translate for scale 2 upsampling).
    """
    U = np.zeros((HH, HL), np.float32)
    for o in range(HH):
        src = o / 2.0 - 0.25
        i0 = int(np.floor(src))
        f = src - i0
        w = {i0: 1.0 - f, i0 + 1: f}
        valid = {i: wi for i, wi in w.items() if 0 <= i < HL and wi > 0}
        tot = sum(valid.values())
        for i, wi in valid.items():
            U[o, i] = wi / tot
    return U


_U = _bilinear_up_matrix()


def _upsample2(x: np.ndarray) -> np.ndarray:
    """(..., 256, 256) -> (..., 512, 512) bilinear, exact jax semantics."""
    lead = x.shape[:-2]
    xf = x.reshape((-1, HL, HL)).astype(np.float32)
    y = np.einsum("yi,nij,xj->nyx", _U, xf, _U, optimize=True)
    return y.reshape(lead + (HH, HH)).astype(np.float32)


def _unfold_hf(x_hr_b: np.ndarray, blur_hr_b: np.ndarray) -> np.ndarray:
    """hf (L, CKK): unfold(x_hr - blur_hr, k=16, s=8), m=(ph,pw), d=(c,i,j)."""
    d = (x_hr_b - blur_hr_b).astype(np.float32)          # (C, 512, 512)
    win = np.lib.stride_tricks.sliding_window_view(d, (K, K), axis=(1, 2))
    win = win[:, ::S, ::S]                                # (C, 63, 63, 16, 16)
    return np.ascontiguousarray(
        win.transpose(1, 2, 0, 3, 4).reshape(L, CKK))


def _fold(cols: np.ndarray) -> np.ndarray:
    """cols (B, CKK, L) -> overlap-add (B, C, 512, 512) (reference col2im)."""
    c6 = cols.reshape(B, C, K, K, NH, NH)
    out = np.zeros((B, C, HH, HH), np.float32)
    for i in range(K):
        for j in range(K):
            out[:, :, i:i + S * NH:S, j:j + S * NH:S] += c6[:, :, i, j]
    return out


_NORM = None


def _norm_map() -> np.ndarray:
    global _NORM
    if _NORM is None:
        _NORM = _fold(np.ones((B, CKK, L), np.float32))
        _NORM = np.maximum(_NORM, 1e-8)
    return _NORM


def _f8(x: np.ndarray) -> np.ndarray:
    import ml_dtypes
    return x.astype(ml_dtypes.float8_e4m3)


# ------------------------------------------------------------ device kernel
_NC = None


def _build_nc(mode: str = "full"):
    """SPMD bass program: rec = attnT.T @ hf, fp8e4m3 DoubleRow matmuls.

    mode: 'full' (the real kernel), 'dma' (input streaming only) or 'mm'
    (matmul schedule off one resident band) — bisect probes for perf work.
    """
    import bass_rust
    import concourse.bass as bass
    import concourse.mybir as mybir
    from concourse.tile import TileContext
    from concourse.vector_clock import ScopedClock

    # Walrus in this build rejects ctrl instructions carrying >2 sem waits;
    # Tile's exit drain waits on every live semaphore.  Split those waits
    # across single-wait drain instructions.
    def _drain_and_barrier(self, tick_clock, wait_clock):
        nc = self.nc
        drain_inst = nc.sync.drain()
        wait_clock.add_sem_waits(
            drain_inst.ins, ScopedClock({None: tick_clock.global_clock}))
        si = drain_inst.ins.sync_info
        waits = list(si.on_wait) if si is not None else []
        if len(waits) > 1:
            drain_inst.ins.sync_info = bass_rust.SyncInfo(
                on_update=list(si.on_update), on_wait=waits[:1])
            for w in waits[1:]:
                d2 = nc.sync.drain()
                d2.ins.sync_info = bass_rust.SyncInfo(on_update=[], on_wait=[w])
        nc.all_engine_barrier()
        popped = nc._tile_sem_poison_stack.pop()
        assert popped is self._sem_poison
        nc.clear_and_free_semaphores(list(self.sems.allocated().values()))
        nc.all_engine_barrier()

    TileContext._drain_and_barrier = _drain_and_barrier

    # Engine sem-name prefix per engine type, for the self-wait post-pass.
    _ENG_SEM = {
        mybir.EngineType.PE: "PE_",
        mybir.EngineType.DVE: "DVE_",
        mybir.EngineType.Activation: "Activation_",
        mybir.EngineType.SP: "SP_",
        mybir.EngineType.Pool: "Pool_",
    }

    band_nops = []       # per-band SP wait-carrier nops, filled at build time
    prelude_nops = []    # (engine, nop) last-resort wait carriers, per context

    def _split_excess_waits(nc):
        """Walrus in this build caps sem waits per instruction (1 for DMA,
        2 otherwise).  Two legal rewrites bring Tile's output under the cap:
          - drop self-engine waits (WAW on a reused slot): engines complete
            in order, so an earlier same-engine producer is already done;
          - hoist remaining excess waits onto the nearest *preceding*
            same-engine instruction with spare capacity — the sequencer
            executes waits in program order, so waiting earlier is strictly
            more conservative.  (Producers of hoisted waits are tile-slot
            reuses >= one full band older, so no deadlock is possible.)
        """
        import bass_rust as _br

        nop_names = {i.ins.name for i in band_nops}
        prelude_by_name = {i.ins.name: i.ins for _, i in prelude_nops}

        def cap(inst):
            # Empirically this walrus accepts at most ONE sem wait per
            # instruction across every struct we hit (DMA, ACT, LW/matmul,
            # ctrl drain).
            return 1

        def set_waits(inst, waits):
            si = inst.sync_info
            ups = list(si.on_update) if si else []
            inst.sync_info = _br.SyncInfo(on_update=ups, on_wait=waits)

        def merge_wait(inst, w):
            """Add wait w to inst, merging same-sem waits by max value."""
            si = inst.sync_info
            waits = list(si.on_wait) if si else []
            for i, ex in enumerate(waits):
                if ex.ant_name == w.ant_name:
                    if w.wait_value > ex.wait_value:
                        waits[i] = w
                    set_waits(inst, waits)
                    return
            set_waits(inst, waits + [w])

        for bb in nc.main_func.blocks:
            cur_nop = None          # most recent band-carrier nop on SP
            streams = {}            # engine -> prior instructions, in order
            bb_preludes = {}        # engine -> prelude nops IN THIS BB only
            for inst in bb.instructions:
                stream = streams.setdefault(inst.engine, [])
                if inst.name in prelude_by_name:
                    bb_preludes.setdefault(inst.engine, []).append(inst)
                    stream.append(inst)
                    continue
                if inst.name in nop_names:
                    cur_nop = inst
                    stream.append(inst)
                    continue
                si = inst.sync_info
                if si is None:
                    stream.append(inst)
                    continue
                waits = list(si.on_wait)
                if len(waits) <= cap(inst):
                    stream.append(inst)
                    continue
                # 1) drop self-engine waits (in-order engines: an earlier
                #    same-engine producer has completed by issue time)
                pfx = _ENG_SEM.get(inst.engine)
                waits = [w for w in waits
                         if not (pfx and w.ant_name.startswith(pfx))]
                # 1b) a WAR wait on the ACT dummy-read is implied by the WAR
                #     wait on the ACT-issued output DMA (same sequencer,
                #     in-order: dummy completed before the DMA was issued)
                if (len(waits) > cap(inst)
                        and any(w.ant_name.startswith("DMAHW") for w in waits)):
                    waits = [w for w in waits
                             if not w.ant_name.startswith("Activation_")]
                if len(waits) > cap(inst):
                    # keep one wait (prefer the DMA-lane RAW for DMAs), hoist
                    # the rest onto earlier same-engine instructions — waits
                    # execute in sequencer program order, so hoisting is
                    # strictly more conservative.  Producers of hoisted waits
                    # are tile-slot reuses from >= 2 pipeline stages earlier,
                    # so a bounded backward hoist cannot deadlock.
                    if type(inst).__name__ == "InstDMACopy":
                        keep = ([w for w in waits if w.ant_name.startswith("DMAHW")]
                                or waits)[:1]
                    else:
                        keep = waits[:1]
                    hoist = [w for w in waits if w not in keep]
                    for w in hoist:
                        placed = False
                        if inst.engine == mybir.EngineType.SP and cur_nop is not None:
                            merge_wait(cur_nop, w)
                            placed = True
                        else:
                            for prior in reversed(stream[-50:]):
                                psi = prior.sync_info
                                pw = list(psi.on_wait) if psi else []
                                if len(pw) < cap(prior):
                                    set_waits(prior, pw + [w])
                                    placed = True
                                    break
                        if not placed:
                            # last resort: prelude nop on this engine (they
                            # sit at the head of this context's stream)
                            for pn in bb_preludes.get(inst.engine, []):
                                psi = pn.sync_info
                                pw = list(psi.on_wait) if psi else []
                                same = [x for x in pw if x.ant_name == w.ant_name]
                                if same or len(pw) < 1:
                                    merge_wait(pn, w)
                                    placed = True
                                    break
                        assert placed, (
                            f"{inst.name}: no carrier for {w.ant_name}")
                    waits = keep
                assert len(waits) <= cap(inst), (
                    f"{inst.name}: still {len(waits)} waits")
                set_waits(inst, waits)
                stream.append(inst)

    dt = mybir.dt
    f32 = dt.float32
    f8 = dt.float8e4
    bf16 = dt.bfloat16
    DR = mybir.MatmulPerfMode.DoubleRow

    nc = bass.Bass(target_bir_lowering=False)
    # attnT8[qb*128 + p, 2c + i, qcol] = attn_scaled[q0 + qb*512 + qcol,
    #                                                c*256 + i*128 + p]
    attnT = nc.dram_tensor("attnT", [N_QB * 128, 2 * NM, QB], f8,
                           kind="ExternalInput")
    # hf8[p, 2c + i, d] = hf[c*256 + i*128 + p, d]
    hf = nc.dram_tensor("hf", [128, 2 * NM, CKK], f8, kind="ExternalInput")
    # rec[p, qb*ND + dt, qcol] = rec_dq[dt*128 + p, qb*512 + qcol]
    rec = nc.dram_tensor("rec", [128, N_QB * ND, QB], bf16,
                         kind="ExternalOutput")
    # Version tag: kernel revisions keep identical I/O signatures, which
    # lets a stale NEFF survive HLO-keyed compile caches.  A [1, VER]
    # input (DMA'd so it isn't pruned) bumps the signature per revision.
    VER = 2
    cfg = nc.dram_tensor("cfg", [1, VER], mybir.dt.uint32,
                         kind="ExternalInput")
    cfg_raw = nc.alloc_sbuf_tensor("cfg_raw", [1, VER], mybir.dt.uint32)

    # hf lives in a raw SBUF tensor so it survives across the TileContexts
    # (loaded once in a prologue context, read-only afterwards).  hf is the
    # STATIONARY operand: out[d, q] accumulates hf_chunk.T @ attn_chunk, so
    # every matmul runs at the ISA maximum (K=256 DoubleRow, M=128, N=512)
    # and the whole contraction is 4 q-blocks x 6 d-tiles x 16 chunks = 384
    # matmuls.
    hf_raw = nc.alloc_sbuf_tensor("hf_raw", [128, 2 * NM, CKK], f8)
    hf_sb = hf_raw.ap()
    # q-block 0's attn band also loads in the prologue (dedicated raw
    # buffer, concurrent with the hf DMA) so the main context's first
    # matmul group isn't serialized behind BOTH loads.
    att0_raw = nc.alloc_sbuf_tensor("att0_raw", [128, 2 * NM, QB], f8)

    with TileContext(nc):
        nc.sync.dma_start(hf_sb[:, :, :], hf[:, :, :])
        nc.sync.dma_start(att0_raw.ap()[:, :, :], attnT[0:128, :, :])
        nc.sync.dma_start(cfg_raw.ap()[:, :], cfg[:, :])

    with TileContext(nc) as tc:
        with (
            tc.tile_pool(name="attp", bufs=2) as attp,
            tc.tile_pool(name="recp", bufs=3) as recp,
            tc.tile_pool(name="psp", bufs=3, space="PSUM") as psp,
        ):
            for eng_name, eng in (("tensor", nc.tensor),
                                  ("vector", nc.vector),
                                  ("scalar", nc.scalar)):
                for i in range(8):
                    prelude_nops.append(
                        (eng.engine,
                         eng.nop(hint=f"prelude_{eng_name}_{i}")))

            last_a = None
            for qb in range(N_QB):
                # SP wait-carrier: absorbs the attn-load WAR waits
                # (band-2 consumers) so each load keeps only its
                # DMA-lane wait.
                band_nops.append(nc.sync.nop(hint=f"band{qb}_carrier"))
                if qb == 0 or (mode == "mm" and last_a is not None):
                    a = last_a = last_a or att0_raw.ap()
                else:
                    a = attp.tile([128, 2 * NM, QB], f8, tag="at")
                    nc.sync.dma_start(
                        a[:, :, :], attnT[qb * 128:(qb + 1) * 128, :, :])
                    last_a = a
                if mode == "dma":
                    continue
                ro = recp.tile([128, ND, QB], bf16, tag="ro")
                for dtp in range(ND // 2):
                    # one 2-bank psum tile per d-tile pair
                    p = psp.tile([128, 2, QB], f32, tag="p")
                    for dt2 in range(2):
                        d0 = (2 * dtp + dt2) * 128
                        for c in range(NM):
                            nc.tensor.matmul(
                                p[:, dt2:dt2 + 1, :],
                                hf_sb[:, 2 * c:2 * c + 2, d0:d0 + 128],
                                a[:, 2 * c:2 * c + 2, :],
                                start=(c == 0), stop=(c == NM - 1),
                                perf_mode=DR)
                    # Each copyback needs a RAW (PE) and a WAR (output DMA)
                    # wait but the ISA takes one per instruction.  The tiny
                    # psum read carries the PE wait (pinned in the schedule
                    # by its RAW dep), so Tile elides the PE wait from the
                    # big cast-copy, which keeps only the WAR wait.
                    tny = recp.tile([128, 1, 1], f32, tag=f"tny{dtp}")
                    nc.vector.tensor_copy(tny[:, :, :], p[:, 0:1, 0:1])
                    nc.vector.tensor_copy(
                        ro[:, 2 * dtp:2 * dtp + 2, :], p[:, :, :])
                # ACT observes the DVE copies via this cheap read (of the
                # LAST copy's output), so the ACT-issued output DMA needs no
                # extra DVE wait of its own (Tile elides observed ticks).
                dmy = recp.tile([128, 1, 1], bf16, tag="dmy")
                nc.scalar.copy(dmy[:, :, :], ro[:, ND - 1:ND, QB - 1:QB])
                nc.scalar.dma_start(
                    rec[:, qb * ND:(qb + 1) * ND, :], ro[:, :, :])
    _split_excess_waits(nc)
    return nc


def _get_nc():
    global _NC
    if _NC is None:
        _NC = _build_nc()
    return _NC


# ---------------------------------------------------------------- benchmark
def bench(in_maps, iters: int = 10):
    """Steady-state per-execution time of the compiled NEFF.

    Re-implements bass2jax.run_bass_via_pjrt's jit/shard_map wrapping, but
    device_puts the inputs once, pre-creates donated output buffers ON
    DEVICE (no host->device zero traffic in the timed region), dispatches
    two batches of executions asynchronously and reports the MARGINAL time
    per execution between them — the fixed dispatch/sync overhead of the
    axon client (~115 ms per batch, measured with an empty NEFF) cancels
    out, leaving the steady-state pipelined per-NEFF device time.
    """
    import time

    import jax
    import jax.numpy as jnp
    import numpy as np
    from jax.experimental.shard_map import shard_map
    from jax.sharding import Mesh, NamedSharding, PartitionSpec

    import concourse.bass2jax as bass2jax
    import concourse.mybir as mybir

    nc = _get_nc()
    bass2jax.install_neuronx_cc_hook()

    in_names, out_names, out_avals, zero_outs = [], [], [], []
    for alloc in nc.m.functions[0].allocations:
        if not isinstance(alloc, mybir.MemoryLocationSet):
            continue
        name = alloc.memorylocations[0].name
        if alloc.kind == "ExternalInput":
            in_names.append(name)
        elif alloc.kind == "ExternalOutput":
            shape = tuple(alloc.tensor_shape)
            dtype = mybir.dt.np(alloc.dtype)
            out_names.append(name)
            out_avals.append(jax.core.ShapedArray(shape, dtype))
            zero_outs.append(np.zeros(shape, dtype))
    n_params = len(in_names)
    n_outs = len(out_avals)
    all_names = in_names + out_names
    donate = tuple(range(n_params, n_params + n_outs))

    def _body(*args):
        outs = bass2jax._bass_exec_p.bind(
            *args,
            out_avals=tuple(out_avals),
            in_names=tuple(all_names),
            out_names=tuple(out_names),
            lowering_input_output_aliases=(),
            sim_require_finite=True,
            sim_require_nnan=True,
            nc=nc,
        )
        return tuple(outs)

    devices = jax.devices()[:N_CORES]
    mesh = Mesh(np.asarray(devices), ("core",))
    sh = NamedSharding(mesh, PartitionSpec("core"))
    sharded = jax.jit(
        shard_map(_body, mesh=mesh,
                  in_specs=(PartitionSpec("core"),) * (n_params + n_outs),
                  out_specs=(PartitionSpec("core"),) * n_outs,
                  check_rep=False),
        donate_argnums=donate, keep_unused=True)

    concat_in = []
    for nm in in_names:
        if nm == "partition_id":
            concat_in.append(
                np.arange(N_CORES, dtype=np.uint32).reshape(N_CORES, 1))
        else:
            concat_in.append(np.concatenate(
                [np.asarray(in_maps[c][nm]) for c in range(N_CORES)], 0))
    dev_in = [jax.device_put(a, sh) for a in concat_in]
    # Donated output buffers created on device — no host->device transfer.
    zeros_fn = jax.jit(
        lambda: tuple(
            jnp.zeros((N_CORES * z.shape[0], *z.shape[1:]), z.dtype)
            for z in zero_outs),
        out_shardings=tuple(sh for _ in zero_outs))

    warm = sharded(*dev_in, *zeros_fn())
    jax.block_until_ready(warm)

    def timed(n):
        zbufs = [zeros_fn() for _ in range(n)]
        jax.block_until_ready(zbufs)
        t0 = time.perf_counter()
        outs = [sharded(*dev_in, *zbufs[i]) for i in range(n)]
        jax.block_until_ready(outs)
        t1 = time.perf_counter()
        del outs
        return t1 - t0

    # Median of repeated two-point marginal estimates: robust both to the
    # fixed per-batch dispatch overhead (cancels in the difference) and to
    # the remote terminal's load fluctuations (median over reps; a min
    # would cherry-pick pairs whose large batch landed in a faster window
    # than their small batch and under-estimate).
    n1, n2 = iters, iters + 80
    margs = []
    for _ in range(9):
        t1 = timed(n1)
        t2 = timed(n2)
        margs.append((t2 - t1) / (n2 - n1))
    per_call_ns = sorted(margs)[len(margs) // 2] * 1e9
    return per_call_ns, warm


# ------------------------------------------------------------------- kernel
def _prepare(x_hr, x_lr_inpainted, attn_map, x_lr_blurred):
    """Host sharding prep: upsample, unfold, quantize, per-core shards."""
    x_hr = np.asarray(x_hr, np.float32)
    x_lr_inpainted = np.asarray(x_lr_inpainted, np.float32)
    attn_map = np.asarray(attn_map, np.float32)
    x_lr_blurred = np.asarray(x_lr_blurred, np.float32)

    blur_hr = _upsample2(x_lr_blurred)                    # (B, C, 512, 512)
    base = _upsample2(x_lr_inpainted)                     # (B, C, 512, 512)

    q_starts = (0, L - LQ)                                # 0 and 1953
    in_maps = []
    hf_pack_cache = {}
    for core in range(N_CORES):
        b, half = core // 2, core % 2
        if b not in hf_pack_cache:
            hfp = np.zeros((MP, CKK), np.float32)
            hfp[:L] = _unfold_hf(x_hr[b], blur_hr[b])
            # [m, d] -> [p, 2c+i, d], m = c*256 + i*128 + p
            hf8 = _f8(hfp).reshape(NM, 2, 128, CKK).transpose(2, 0, 1, 3)
            hf_pack_cache[b] = np.ascontiguousarray(
                hf8.reshape(128, 2 * NM, CKK))
        q0 = q_starts[half]
        atq = np.zeros((MP, LQP), _f8(np.zeros(1)).dtype)
        atq[:L, :LQ] = _f8(attn_map[b, 0, q0:q0 + LQ, :] * SCALE).T
        # [m, q] -> [qb*128 + p, 2c+i, qcol], m = c*256+i*128+p, q = qb*512+qcol
        at = atq.reshape(NM, 2, 128, N_QB, QB).transpose(3, 2, 0, 1, 4)
        at = np.ascontiguousarray(at.reshape(N_QB * 128, 2 * NM, QB))
        in_maps.append({"attnT": at, "hf": hf_pack_cache[b],
                        "cfg": np.zeros((1, 2), np.uint32)})
    return in_maps, base


def _rec_dq(rec_core: np.ndarray) -> np.ndarray:
    """Device rec [128, N_QB*ND, 512] bf16 -> [768 d, 2048 q] fp32."""
    r = np.asarray(rec_core, np.float32).reshape(128, N_QB, ND, QB)
    return r.transpose(2, 0, 1, 3).reshape(CKK, LQP)


def _finish(per_core_rec, base):
    """Gather: stitch q-halves, fold, de-scale + normalize, add base."""
    cols = np.empty((B, CKK, L), np.float32)
    for b in range(B):
        rec_a = _rec_dq(per_core_rec[2 * b])                   # (768, 2048)
        rec_b = _rec_dq(per_core_rec[2 * b + 1])
        cols[b, :, :LQ] = rec_a[:, :LQ]
        cols[b, :, LQ:] = rec_b[:, 2 * LQ - L:LQ]
    img = _fold(cols)
    out = base + img / (_norm_map() * SCALE)
    return out.astype(np.float32)


def kernel(x_hr, x_lr_inpainted, attn_map, x_lr_blurred):
    global LAST_RESULT
    from concourse.bass_utils import run_bass_kernel_spmd

    in_maps, base = _prepare(x_hr, x_lr_inpainted, attn_map, x_lr_blurred)
    nc = _get_nc()
    trace = bool(os.environ.get("KERNEL_TRACE"))
    res = run_bass_kernel_spmd(nc, in_maps, list(range(N_CORES)), trace=trace)
    LAST_RESULT = res
    return _finish([res.results[c]["rec"] for c in range(N_CORES)], base)


# revision 26
# speedup vs baseline: 1.3528x; 1.0291x over previous
"""AttentionUpscaling Trainium2 kernel.

Strategy (8 NeuronCores):
  - Pure data parallelism over batch (4) x query-half (2): each core owns one
    (batch, q-half) shard of the L x L attention matmul (the ~97 GFLOP that
    dominate this problem).
  - Host side (sharding prep): bilinear 2x upsample (exact jax semantics via a
    sparse banded matrix), unfold of the high-frequency residual, fp8e4m3
    quantization (attn pre-scaled by 2^17 into the fp8 sweet spot), and
    device-layout packing so every band is ONE contiguous-per-partition DMA.
  - Device side (SPMD bass/Tile program, same NEFF on all 8 cores):
    rec[q, d] = sum_m attnT[m, q] * hf[m, d]; contraction over m=4096 in 16
    DoubleRow chunks of 256 (fp8e4m3 at 2x PE rate), hf SBUF-resident (one
    3 MB DMA), attnT streamed in 256-column bands (one 1 MB DMA per band,
    double-buffered), PSUM accumulation, DVE copyback with fused fp32->bf16
    cast, HWDGE DMA out (bf16 halves the output traffic).
  - Host side (gather): overlap-add fold + overlap-count normalization (the
    2^17 attn pre-scale is folded into the normalization divide) + base image
    add, then stitch the two q-halves per batch.
"""

import os

import numpy as np

# ---------------------------------------------------------------- constants
B, C = 4, 3
HH = 512          # HR height/width
HL = 256          # LR height/width
K = 16            # HR patch size
S = 8             # HR stride
NH = (HH - K) // S + 1          # 63 patches per axis
L = NH * NH                     # 3969 patches
CKK = C * K * K                 # 768
NPH = 32                        # patch-rows per core (ph 0..31 / 31..62)
LQ = NPH * NH                   # 2016 q rows per core
LQP = 2048                      # padded q rows (16 x 128)
MP = 4096                       # padded contraction dim (16 x 256)
N_CORES = 8
NM = 16                         # 256-row DoubleRow contraction chunks
ND = CKK // 128                 # 6 stationary hf d-tiles of 128
QB = 512                        # attnT columns per band (= max psum free dim)
N_QB = LQP // QB                # 4 q-blocks
SCALE = float(2.0 ** 17)        # attn fp8 pre-scale (max ~137 < 240 = e4m3 max)

LAST_RESULT = None              # BassKernelResults of the most recent run


# ------------------------------------------------------------- host helpers
def _bilinear_up_matrix() -> np.ndarray:
    """U (512, 256): exact jax.image.resize 'bilinear' 256->512 upsample.

    Half-pixel centers: src(o) = o/2 - 0.25; triangle weights, renormalized
    at the edges (matches jax's scale_and_translate for scale 2 upsampling).
    """
    U = np.zeros((HH, HL), np.float32)
    for o in range(HH):
        src = o / 2.0 - 0.25
        i0 = int(np.floor(src))
        f = src - i0
        w = {i0: 1.0 - f, i0 + 1: f}
        valid = {i: wi for i, wi in w.items() if 0 <= i < HL and wi > 0}
        tot = sum(valid.values())
        for i, wi in valid.items():
            U[o, i] = wi / tot
    return U


_U = _bilinear_up_matrix()


def _upsample2(x: np.ndarray) -> np.ndarray:
    """(..., 256, 256) -> (..., 512, 512) bilinear, exact jax semantics."""
    lead = x.shape[:-2]
    xf = x.reshape((-1, HL, HL)).astype(np.float32)
    y = np.einsum("yi,nij,xj->nyx", _U, xf, _U, optimize=True)
    return y.reshape(lead + (HH, HH)).astype(np.float32)


def _unfold_hf(x_hr_b: np.ndarray, blur_hr_b: np.ndarray) -> np.ndarray:
    """hf (L, CKK): unfold(x_hr - blur_hr, k=16, s=8), m=(ph,pw), d=(c,i,j)."""
    d = (x_hr_b - blur_hr_b).astype(np.float32)          # (C, 512, 512)
    win = np.lib.stride_tricks.sliding_window_view(d, (K, K), axis=(1, 2))
    win = win[:, ::S, ::S]                                # (C, 63, 63, 16, 16)
    return np.ascontiguousarray(
        win.transpose(1, 2, 0, 3, 4).reshape(L, CKK))


def _fold(cols: np.ndarray) -> np.ndarray:
    """cols (B, CKK, L) -> overlap-add (B, C, 512, 512) (reference col2im)."""
    c6 = cols.reshape(B, C, K, K, NH, NH)
    out = np.zeros((B, C, HH, HH), np.float32)
    for i in range(K):
        for j in range(K):
            out[:, :, i:i + S * NH:S, j:j + S * NH:S] += c6[:, :, i, j]
    return out


_NORM = None


def _norm_map() -> np.ndarray:
    global _NORM
    if _NORM is None:
        _NORM = _fold(np.ones((B, CKK, L), np.float32))
        _NORM = np.maximum(_NORM, 1e-8)
    return _NORM


def _f8(x: np.ndarray) -> np.ndarray:
    import ml_dtypes
    return x.astype(ml_dtypes.float8_e4m3)


# ------------------------------------------------------------ device kernel
_NC = None


def _build_nc(mode: str = "full"):
    """SPMD bass program: rec = attnT.T @ hf, fp8e4m3 DoubleRow matmuls.

    mode: 'full' (the real kernel), 'dma' (input streaming only) or 'mm'
    (matmul schedule off one resident band) — bisect probes for perf work.
    """
    import bass_rust
    import concourse.bass as bass
    import concourse.mybir as mybir
    from concourse.tile import TileContext
    from concourse.vector_clock import ScopedClock

    # Walrus in this build rejects ctrl instructions carrying >2 sem waits;
    # Tile's exit drain waits on every live semaphore.  Split those waits
    # across single-wait drain instructions.
    def _drain_and_barrier(self, tick_clock, wait_clock):
        nc = self.nc
        drain_inst = nc.sync.drain()
        wait_clock.add_sem_waits(
            drain_inst.ins, ScopedClock({None: tick_clock.global_clock}))
        si = drain_inst.ins.sync_info
        waits = list(si.on_wait) if si is not None else []
        if len(waits) > 1:
            drain_inst.ins.sync_info = bass_rust.SyncInfo(
                on_update=list(si.on_update), on_wait=waits[:1])
            for w in waits[1:]:
                d2 = nc.sync.drain()
                d2.ins.sync_info = bass_rust.SyncInfo(on_update=[], on_wait=[w])
        nc.all_engine_barrier()
        popped = nc._tile_sem_poison_stack.pop()
        assert popped is self._sem_poison
        nc.clear_and_free_semaphores(list(self.sems.allocated().values()))
        nc.all_engine_barrier()

    TileContext._drain_and_barrier = _drain_and_barrier

    # Engine sem-name prefix per engine type, for the self-wait post-pass.
    _ENG_SEM = {
        mybir.EngineType.PE: "PE_",
        mybir.EngineType.DVE: "DVE_",
        mybir.EngineType.Activation: "Activation_",
        mybir.EngineType.SP: "SP_",
        mybir.EngineType.Pool: "Pool_",
    }

    band_nops = []       # per-band SP wait-carrier nops, filled at build time
    prelude_nops = []    # (engine, nop) last-resort wait carriers, per context

    def _split_excess_waits(nc):
        """Walrus in this build caps sem waits per instruction (1 for DMA,
        2 otherwise).  Two legal rewrites bring Tile's output under the cap:
          - drop self-engine waits (WAW on a reused slot): engines complete
            in order, so an earlier same-engine producer is already done;
          - hoist remaining excess waits onto the nearest *preceding*
            same-engine instruction with spare capacity — the sequencer
            executes waits in program order, so waiting earlier is strictly
            more conservative.  (Producers of hoisted waits are tile-slot
            reuses >= one full band older, so no deadlock is possible.)
        """
        import bass_rust as _br

        nop_names = {i.ins.name for i in band_nops}
        prelude_by_name = {i.ins.name: i.ins for _, i in prelude_nops}

        def cap(inst):
            # Empirically this walrus accepts at most ONE sem wait per
            # instruction across every struct we hit (DMA, ACT, LW/matmul,
            # ctrl drain).
            return 1

        def set_waits(inst, waits):
            si = inst.sync_info
            ups = list(si.on_update) if si else []
            inst.sync_info = _br.SyncInfo(on_update=ups, on_wait=waits)

        def merge_wait(inst, w):
            """Add wait w to inst, merging same-sem waits by max value."""
            si = inst.sync_info
            waits = list(si.on_wait) if si else []
            for i, ex in enumerate(waits):
                if ex.ant_name == w.ant_name:
                    if w.wait_value > ex.wait_value:
                        waits[i] = w
                    set_waits(inst, waits)
                    return
            set_waits(inst, waits + [w])

        def merge_to_nops(nops, w):
            """Place wait w on one of the band-carrier nops: merge into a
            same-sem wait by max, else the first empty nop (walrus accepts
            at most ONE sem wait per ctrl instruction)."""
            for n in nops:
                si = n.sync_info
                if si and any(x.ant_name == w.ant_name for x in si.on_wait):
                    merge_wait(n, w)
                    return True
            for n in nops:
                si = n.sync_info
                if not (si and list(si.on_wait)):
                    merge_wait(n, w)
                    return True
            return False

        for bb in nc.main_func.blocks:
            cur_nops = []           # current band's carrier nops on SP
            streams = {}            # engine -> prior instructions, in order
            bb_preludes = {}        # engine -> prelude nops IN THIS BB only
            for inst in bb.instructions:
                stream = streams.setdefault(inst.engine, [])
                if inst.name in prelude_by_name:
                    bb_preludes.setdefault(inst.engine, []).append(inst)
                    stream.append(inst)
                    continue
                if inst.name in nop_names:
                    cur_nops = (cur_nops + [inst])[-3:]
                    stream.append(inst)
                    continue
                si = inst.sync_info
                if si is None:
                    stream.append(inst)
                    continue
                waits = list(si.on_wait)
                if len(waits) <= cap(inst):
                    stream.append(inst)
                    continue
                # 1) drop self-engine waits (in-order engines: an earlier
                #    same-engine producer has completed by issue time)
                pfx = _ENG_SEM.get(inst.engine)
                waits = [w for w in waits
                         if not (pfx and w.ant_name.startswith(pfx))]
                # 1b) a WAR wait on the ACT dummy-read is implied by the WAR
                #     wait on the ACT-issued output DMA (same sequencer,
                #     in-order: dummy completed before the DMA was issued)
                if (len(waits) > cap(inst)
                        and any(w.ant_name.startswith("DMAHW") for w in waits)):
                    waits = [w for w in waits
                             if not w.ant_name.startswith("Activation_")]
                if len(waits) > cap(inst):
                    # keep one wait (prefer the DMA-lane RAW for DMAs), hoist
                    # the rest onto earlier same-engine instructions — waits
                    # execute in sequencer program order, so hoisting is
                    # strictly more conservative.  Producers of hoisted waits
                    # are tile-slot reuses from >= 2 pipeline stages earlier,
                    # so a bounded backward hoist cannot deadlock.
                    if type(inst).__name__ == "InstDMACopy":
                        keep = ([w for w in waits if w.ant_name.startswith("DMAHW")]
                                or waits)[:1]
                    else:
                        keep = waits[:1]
                    hoist = [w for w in waits if w not in keep]
                    for w in hoist:
                        placed = False
                        if inst.engine == mybir.EngineType.SP and cur_nops:
                            placed = merge_to_nops(cur_nops, w)
                        if not placed:
                            for prior in reversed(stream[-50:]):
                                psi = prior.sync_info
                                pw = list(psi.on_wait) if psi else []
                                if len(pw) < cap(prior):
                                    set_waits(prior, pw + [w])
                                    placed = True
                                    break
                        if not placed:
                            # last resort: prelude nop on this engine (they
                            # sit at the head of this context's stream)
                            for pn in bb_preludes.get(inst.engine, []):
                                psi = pn.sync_info
                                pw = list(psi.on_wait) if psi else []
                                same = [x for x in pw if x.ant_name == w.ant_name]
                                if same or len(pw) < 1:
                                    merge_wait(pn, w)
                                    placed = True
                                    break
                        assert placed, (
                            f"{inst.name}: no carrier for {w.ant_name}")
                    waits = keep
                assert len(waits) <= cap(inst), (
                    f"{inst.name}: still {len(waits)} waits")
                set_waits(inst, waits)
                stream.append(inst)

    dt = mybir.dt
    f32 = dt.float32
    f8 = dt.float8e4
    bf16 = dt.bfloat16
    DR = mybir.MatmulPerfMode.DoubleRow

    nc = bass.Bass(target_bir_lowering=False)
    # attnT8[qb*128 + p, 2c + i, qcol] = attn_scaled[q0 + qb*512 + qcol,
    #                                                c*256 + i*128 + p]
    attnT = nc.dram_tensor("attnT", [N_QB * 128, 2 * NM, QB], f8,
                           kind="ExternalInput")
    # hf8[p, 2c + i, d] = hf[c*256 + i*128 + p, d]
    hf = nc.dram_tensor("hf", [128, 2 * NM, CKK], f8, kind="ExternalInput")
    # rec[p, qb*ND + dt, qcol] = rec_dq[dt*128 + p, qb*512 + qcol]
    rec = nc.dram_tensor("rec", [128, N_QB * ND, QB], bf16,
                         kind="ExternalOutput")
    # Version tag: kernel revisions keep identical I/O signatures, which
    # lets a stale NEFF survive HLO-keyed compile caches.  A [1, VER]
    # input (DMA'd so it isn't pruned) bumps the signature per revision.
    VER = 3
    cfg = nc.dram_tensor("cfg", [1, VER], mybir.dt.uint32,
                         kind="ExternalInput")
    cfg_raw = nc.alloc_sbuf_tensor("cfg_raw", [1, VER], mybir.dt.uint32)

    # hf is the STATIONARY operand: out[d, q] accumulates hf_chunk.T @
    # attn_chunk, so every matmul runs at the ISA maximum (K=256 DoubleRow,
    # M=128, N=512) and the whole contraction is 4 q-blocks x 6 d-tiles x
    # 16 chunks = 384 matmuls.
    #
    # hf and q-block 0's attn band load in QUARTER chunks as tile-tracked
    # DMAs interleaved at the head of the ONE TileContext: the first matmul
    # group starts as soon as quarter 0 lands instead of idling behind a
    # prologue-context barrier for the full 5.1 MB (the sim trace showed
    # an 18.4 us PE startup hole).
    NQRT = 4
    CQ = NM // NQRT                 # contraction chunks per quarter

    with TileContext(nc) as tc:
        with (
            tc.tile_pool(name="stag", bufs=1) as stag,
            tc.tile_pool(name="attp", bufs=2) as attp,
            tc.tile_pool(name="recp", bufs=3) as recp,
            tc.tile_pool(name="psp", bufs=3, space="PSUM") as psp,
        ):
            for eng_name, eng in (("tensor", nc.tensor),
                                  ("vector", nc.vector),
                                  ("scalar", nc.scalar)):
                for i in range(8):
                    prelude_nops.append(
                        (eng.engine,
                         eng.nop(hint=f"prelude_{eng_name}_{i}")))

            nc.sync.dma_start(cfg_raw.ap()[:, :], cfg[:, :])
            att0q, hfq = [], []
            for q in range(NQRT):
                aq = stag.tile([128, 2 * CQ, QB], f8, tag=f"a0q{q}")
                nc.sync.dma_start(
                    aq[:, :, :],
                    attnT[0:128, 2 * CQ * q:2 * CQ * (q + 1), :])
                att0q.append(aq)
                hq = stag.tile([128, 2 * CQ, CKK], f8, tag=f"hfq{q}")
                nc.sync.dma_start(
                    hq[:, :, :], hf[:, 2 * CQ * q:2 * CQ * (q + 1), :])
                hfq.append(hq)

            last_a = None
            for qb in range(N_QB):
                # SP wait-carrier: absorbs the attn-load WAR waits
                # (band-2 consumers) so each load keeps only its
                # DMA-lane wait.
                for j in range(3):
                    band_nops.append(
                        nc.sync.nop(hint=f"band{qb}_carrier{j}"))
                if qb == 0 or (mode == "mm" and last_a is not None):
                    a = last_a = last_a or att0q
                else:
                    a = attp.tile([128, 2 * NM, QB], f8, tag="at")
                    nc.sync.dma_start(
                        a[:, :, :], attnT[qb * 128:(qb + 1) * 128, :, :])
                    last_a = a
                if mode == "dma":
                    continue
                for dtp in range(ND // 2):
                    # one 2-bank psum tile per d-tile pair
                    p = psp.tile([128, 2, QB], f32, tag="p")
                    for dt2 in range(2):
                        d0 = (2 * dtp + dt2) * 128
                        for c in range(NM):
                            cq, ci = divmod(c, CQ)
                            if isinstance(a, list):
                                rhs = a[cq][:, 2 * ci:2 * ci + 2, :]
                            else:
                                rhs = a[:, 2 * c:2 * c + 2, :]
                            nc.tensor.matmul(
                                p[:, dt2:dt2 + 1, :],
                                hfq[cq][:, 2 * ci:2 * ci + 2, d0:d0 + 128],
                                rhs,
                                start=(c == 0), stop=(c == NM - 1),
                                perf_mode=DR)
                    # Each copyback needs a RAW (PE) and a WAR (output DMA)
                    # wait but the ISA takes one per instruction.  The tiny
                    # psum read carries the PE wait (pinned in the schedule
                    # by its RAW dep), so Tile elides the PE wait from the
                    # big cast-copy, which keeps only the WAR wait.
                    tny = recp.tile([128, 1, 1], f32, tag=f"tny{dtp}")
                    nc.vector.tensor_copy(tny[:, :, :], p[:, 0:1, 0:1])
                    # Per-pair ro tile: exactly ONE copy writer and ONE DMA
                    # reader each, so every instruction stays within the
                    # 1-wait cap after elision (a shared whole-band ro tile
                    # with 3 DMA readers deadlocked the ring in CoreSim).
                    # Per-pair output DMAs also shrink the kernel tail to
                    # the LAST pair's copy + one small DMA.
                    ro = recp.tile([128, 2, QB], bf16, tag=f"ro{dtp}")
                    nc.vector.tensor_copy(ro[:, :, :], p[:, :, :])
                    # ACT observes the DVE copy via this cheap read, so the
                    # ACT-issued output DMA needs no extra DVE wait of its
                    # own (Tile elides observed ticks).
                    dmy = recp.tile([128, 1, 1], bf16, tag=f"dmy{dtp}")
                    nc.scalar.copy(dmy[:, :, :], ro[:, 1:2, QB - 1:QB])
                    nc.scalar.dma_start(
                        rec[:, qb * ND + 2 * dtp:qb * ND + 2 * dtp + 2, :],
                        ro[:, :, :])
    _split_excess_waits(nc)
    return nc


def _get_nc():
    global _NC
    if _NC is None:
        _NC = _build_nc()
    return _NC


# ---------------------------------------------------------------- benchmark
def bench(in_maps, iters: int = 10):
    """Steady-state per-execution time of the compiled NEFF.

    Re-implements bass2jax.run_bass_via_pjrt's jit/shard_map wrapping, but
    device_puts the inputs once, pre-creates donated output buffers ON
    DEVICE (no host->device zero traffic in the timed region), dispatches
    two batches of executions asynchronously and reports the MARGINAL time
    per execution between them — the fixed dispatch/sync overhead of the
    axon client (~115 ms per batch, measured with an empty NEFF) cancels
    out, leaving the steady-state pipelined per-NEFF device time.
    """
    import time

    import jax
    import jax.numpy as jnp
    import numpy as np
    from jax.experimental.shard_map import shard_map
    from jax.sharding import Mesh, NamedSharding, PartitionSpec

    import concourse.bass2jax as bass2jax
    import concourse.mybir as mybir

    nc = _get_nc()
    bass2jax.install_neuronx_cc_hook()

    in_names, out_names, out_avals, zero_outs = [], [], [], []
    for alloc in nc.m.functions[0].allocations:
        if not isinstance(alloc, mybir.MemoryLocationSet):
            continue
        name = alloc.memorylocations[0].name
        if alloc.kind == "ExternalInput":
            in_names.append(name)
        elif alloc.kind == "ExternalOutput":
            shape = tuple(alloc.tensor_shape)
            dtype = mybir.dt.np(alloc.dtype)
            out_names.append(name)
            out_avals.append(jax.core.ShapedArray(shape, dtype))
            zero_outs.append(np.zeros(shape, dtype))
    n_params = len(in_names)
    n_outs = len(out_avals)
    all_names = in_names + out_names
    donate = tuple(range(n_params, n_params + n_outs))

    def _body(*args):
        outs = bass2jax._bass_exec_p.bind(
            *args,
            out_avals=tuple(out_avals),
            in_names=tuple(all_names),
            out_names=tuple(out_names),
            lowering_input_output_aliases=(),
            sim_require_finite=True,
            sim_require_nnan=True,
            nc=nc,
        )
        return tuple(outs)

    devices = jax.devices()[:N_CORES]
    mesh = Mesh(np.asarray(devices), ("core",))
    sh = NamedSharding(mesh, PartitionSpec("core"))
    sharded = jax.jit(
        shard_map(_body, mesh=mesh,
                  in_specs=(PartitionSpec("core"),) * (n_params + n_outs),
                  out_specs=(PartitionSpec("core"),) * n_outs,
                  check_rep=False),
        donate_argnums=donate, keep_unused=True)

    concat_in = []
    for nm in in_names:
        if nm == "partition_id":
            concat_in.append(
                np.arange(N_CORES, dtype=np.uint32).reshape(N_CORES, 1))
        else:
            concat_in.append(np.concatenate(
                [np.asarray(in_maps[c][nm]) for c in range(N_CORES)], 0))
    dev_in = [jax.device_put(a, sh) for a in concat_in]
    # Donated output buffers created on device — no host->device transfer.
    zeros_fn = jax.jit(
        lambda: tuple(
            jnp.zeros((N_CORES * z.shape[0], *z.shape[1:]), z.dtype)
            for z in zero_outs),
        out_shardings=tuple(sh for _ in zero_outs))

    warm = sharded(*dev_in, *zeros_fn())
    jax.block_until_ready(warm)

    def timed(n):
        zbufs = [zeros_fn() for _ in range(n)]
        jax.block_until_ready(zbufs)
        t0 = time.perf_counter()
        outs = [sharded(*dev_in, *zbufs[i]) for i in range(n)]
        jax.block_until_ready(outs)
        t1 = time.perf_counter()
        del outs
        return t1 - t0

    # Median of repeated two-point marginal estimates: robust both to the
    # fixed per-batch dispatch overhead (cancels in the difference) and to
    # the remote terminal's load fluctuations (median over reps; a min
    # would cherry-pick pairs whose large batch landed in a faster window
    # than their small batch and under-estimate).
    n1, n2 = iters, iters + 80
    margs = []
    for _ in range(9):
        t1 = timed(n1)
        t2 = timed(n2)
        margs.append((t2 - t1) / (n2 - n1))
    per_call_ns = sorted(margs)[len(margs) // 2] * 1e9
    return per_call_ns, warm


# ------------------------------------------------------------------- kernel
def _prepare(x_hr, x_lr_inpainted, attn_map, x_lr_blurred):
    """Host sharding prep: upsample, unfold, quantize, per-core shards."""
    x_hr = np.asarray(x_hr, np.float32)
    x_lr_inpainted = np.asarray(x_lr_inpainted, np.float32)
    attn_map = np.asarray(attn_map, np.float32)
    x_lr_blurred = np.asarray(x_lr_blurred, np.float32)

    blur_hr = _upsample2(x_lr_blurred)                    # (B, C, 512, 512)
    base = _upsample2(x_lr_inpainted)                     # (B, C, 512, 512)

    q_starts = (0, L - LQ)                                # 0 and 1953
    in_maps = []
    hf_pack_cache = {}
    for core in range(N_CORES):
        b, half = core // 2, core % 2
        if b not in hf_pack_cache:
            hfp = np.zeros((MP, CKK), np.float32)
            hfp[:L] = _unfold_hf(x_hr[b], blur_hr[b])
            # [m, d] -> [p, 2c+i, d], m = c*256 + i*128 + p
            hf8 = _f8(hfp).reshape(NM, 2, 128, CKK).transpose(2, 0, 1, 3)
            hf_pack_cache[b] = np.ascontiguousarray(
                hf8.reshape(128, 2 * NM, CKK))
        q0 = q_starts[half]
        atq = np.zeros((MP, LQP), _f8(np.zeros(1)).dtype)
        atq[:L, :LQ] = _f8(attn_map[b, 0, q0:q0 + LQ, :] * SCALE).T
        # [m, q] -> [qb*128 + p, 2c+i, qcol], m = c*256+i*128+p, q = qb*512+qcol
        at = atq.reshape(NM, 2, 128, N_QB, QB).transpose(3, 2, 0, 1, 4)
        at = np.ascontiguousarray(at.reshape(N_QB * 128, 2 * NM, QB))
        in_maps.append({"attnT": at, "hf": hf_pack_cache[b],
                        "cfg": np.zeros((1, 3), np.uint32)})
    return in_maps, base


def _rec_dq(rec_core: np.ndarray) -> np.ndarray:
    """Device rec [128, N_QB*ND, 512] bf16 -> [768 d, 2048 q] fp32."""
    r = np.asarray(rec_core, np.float32).reshape(128, N_QB, ND, QB)
    return r.transpose(2, 0, 1, 3).reshape(CKK, LQP)


def _finish(per_core_rec, base):
    """Gather: stitch q-halves, fold, de-scale + normalize, add base."""
    cols = np.empty((B, CKK, L), np.float32)
    for b in range(B):
        rec_a = _rec_dq(per_core_rec[2 * b])                   # (768, 2048)
        rec_b = _rec_dq(per_core_rec[2 * b + 1])
        cols[b, :, :LQ] = rec_a[:, :LQ]
        cols[b, :, LQ:] = rec_b[:, 2 * LQ - L:LQ]
    img = _fold(cols)
    out = base + img / (_norm_map() * SCALE)
    return out.astype(np.float32)


def kernel(x_hr, x_lr_inpainted, attn_map, x_lr_blurred):
    global LAST_RESULT
    from concourse.bass_utils import run_bass_kernel_spmd

    in_maps, base = _prepare(x_hr, x_lr_inpainted, attn_map, x_lr_blurred)
    nc = _get_nc()
    trace = bool(os.environ.get("KERNEL_TRACE"))
    res = run_bass_kernel_spmd(nc, in_maps, list(range(N_CORES)), trace=trace)
    LAST_RESULT = res
    return _finish([res.results[c]["rec"] for c in range(N_CORES)], base)
